# revision 1
# baseline (speedup 1.0000x reference)
"""Trainium2 Bass kernel for nn_AttentionMechanisms_1365799600322.

Reference computation (B=4, S=4096, HID=2048, H=16, D=128):
    q = x@Wq+bq; k = x@Wk+bk; v = x@Wv+bv          (reshaped [B,S,H,D])
    scores[b,s,h,g] = q[b,s,h,:]@k[b,s,g,:] * c_scale/sqrt(D)   # per-token HxH
    w = softmax(scores, -1); attn = w@v; out = attn@Wo + bo

Every op mixes only within a token, so we shard the B*S=16384 tokens
across 8 NeuronCores (2048 tokens/core) with zero collectives.

Per-core pipeline (bf16 matmul inputs, f32 PSUM accumulation):
  xT [din, tok] passed transposed from host; weights pre-tiled on host so
  every weight-strip DMA is 4KB-contiguous per partition.
  qT/kT [d, t*16+h]   <- lhsT=W block, rhs=xT (transposed out, token-major
                         interleave so each 8-token group is one contiguous
                         128-col matmul operand)
  v     [tok, dout]   <- lhsT=xT block, rhs=Wv strip (natural out)
  v_grp [(t,g), d]    <- DRAM bounce (partition remap in DRAM-side APs)
  s_T [(t,g),(t',h)]  <- lhsT=kT group, rhs=qT group (one MM per group)
  p = exp(scale*s) * mask01   (ScalarE exp + DVE bf16 4x multiply; the 0/1
                               mask kills cross-token terms)
  rowsum <- all-ones stationary matmul (broadcasts col-sums to all parts)
  attn_T [d,(t',h)] <- lhsT=v_grp, rhs=p; normalized by reciprocal(rowsum)
                       fused into the PSUM->SBUF copy
  outT [dout, tok]  <- lhsT=Wo block, rhs=attn_T stride-16 head slice
  All four biases enter as K=1 rank-1 matmuls appended to the PSUM
  accumulations (ones-row x bias-row), so no per-partition bias ops.
"""

import numpy as np
import ml_dtypes

import concourse.bass as bass
import concourse.mybir as mybir
from concourse.tile import TileContext

BF16 = mybir.dt.bfloat16
F32 = mybir.dt.float32

B, S, HID, H = 4, 4096, 2048, 16
D = HID // H            # 128
T_TOT = B * S           # 16384
NCORES = 8
T = T_TOT // NCORES     # 2048 tokens per core
KT = HID // 128         # 16 contraction tiles
FT = HID // 128         # 16 feature tiles (== heads for q/k/v layouts)
CHUNK = 512
NCHUNK = T // CHUNK     # 4
NSUB = CHUNK // 128     # 4 token-subtiles per chunk
GRP = 8                 # tokens per attention group
NGRP = CHUNK // GRP     # 64 groups per chunk
QUAD = 4                # groups per psum batch
NBATCH = NGRP // QUAD   # 16


def build_nc(with_bias: bool = False):
    nc = bass.Bass()

    xT = nc.declare_dram_parameter("xT", [HID, T], BF16, isOutput=False)
    # pre-tiled: w_h[f*128+p, kt*128+m] = W[kt*128+p, f*128+m]
    wq = nc.declare_dram_parameter("wq", [HID, HID], BF16, isOutput=False)
    wk = nc.declare_dram_parameter("wk", [HID, HID], BF16, isOutput=False)
    wo = nc.declare_dram_parameter("wo", [HID, HID], BF16, isOutput=False)
    wv = nc.declare_dram_parameter("wv", [HID, HID], BF16, isOutput=False)  # natural
    brows = nc.declare_dram_parameter("brows", [1, 4 * HID], BF16, isOutput=False)
    scv = nc.declare_dram_parameter("scv", [128, 1], F32, isOutput=False)
    mask01 = nc.declare_dram_parameter("mask01", [128, 512], BF16, isOutput=False)
    ones = nc.declare_dram_parameter("ones", [128, 512], BF16, isOutput=False)
    outT = nc.declare_dram_parameter("outT", [HID, T], F32, isOutput=True)

    xT_r = xT[:, :].rearrange("(kt p) t -> p kt t", p=128)       # [128,16,T]
    outT_r = outT[:, :].rearrange("(f p) t -> p f t", p=128)     # [128,16,T]
    wv_r = wv[:, :].rearrange("(kt p) n -> p kt n", p=128)       # [128,16,2048]

    with TileContext(nc) as tc:
        from contextlib import ExitStack

        with ExitStack() as _es:
            cpool = _es.enter_context(tc.tile_pool(name="const", bufs=1))
            xpool = _es.enter_context(tc.tile_pool(name="xin", bufs=2))
            qpool = _es.enter_context(tc.tile_pool(name="qq", bufs=2))
            kpool = _es.enter_context(tc.tile_pool(name="kk", bufs=1))
            wpool = _es.enter_context(tc.tile_pool(name="wstrip", bufs=4))
            wvpool = _es.enter_context(tc.tile_pool(name="wvtile", bufs=18))
            vnpool = _es.enter_context(tc.tile_pool(name="vnat", bufs=4))
            vgpool = _es.enter_context(tc.tile_pool(name="vgrp", bufs=1))
            vdpool = _es.enter_context(tc.tile_pool(name="vdram", bufs=2, space="DRAM"))
            atpool = _es.enter_context(tc.tile_pool(name="attnT", bufs=1))
            ppool = _es.enter_context(tc.tile_pool(name="pexp", bufs=3))
            rpool = _es.enter_context(tc.tile_pool(name="rinv", bufs=3))
            opool = _es.enter_context(tc.tile_pool(name="ostage", bufs=3))
            ppsum = _es.enter_context(tc.tile_pool(name="pproj", bufs=2, space="PSUM"))
            vpsum = _es.enter_context(tc.tile_pool(name="pv", bufs=2, space="PSUM"))
            spsum = _es.enter_context(tc.tile_pool(name="psc", bufs=2, space="PSUM"))
            rpsum = _es.enter_context(tc.tile_pool(name="prs", bufs=1, space="PSUM"))
            apsum = _es.enter_context(tc.tile_pool(name="pat", bufs=1, space="PSUM"))
            # ---------------- constants ----------------
            br_sb = cpool.tile([1, 4 * HID], BF16, tag="br")   # bq,bk,bv,bo rows
            sc_sb = cpool.tile([128, 1], F32, tag="sc")
            mk_sb = cpool.tile([128, 512], BF16, tag="mask")
            ones_sb = cpool.tile([128, 512], BF16, tag="ones")
            nc.sync.dma_start(out=br_sb[:], in_=brows[:, :])
            nc.sync.dma_start(out=sc_sb[:], in_=scv[:, :])
            nc.sync.dma_start(out=mk_sb[:], in_=mask01[:, :])
            nc.sync.dma_start(out=ones_sb[:], in_=ones[:, :])
            one_row = ones_sb[0:1, :]            # [1, 512] of ones
            ones_sq = ones_sb[:, 0:128]          # [128, 128] of ones

            for c in range(NCHUNK):
                tok0 = c * CHUNK
                # ---------- load xT chunk [128, (kt,512)] ----------
                x_sb = xpool.tile([128, KT * CHUNK], BF16, tag="x")
                nc.sync.dma_start(
                    out=x_sb[:].rearrange("p (kt t) -> p kt t", t=CHUNK),
                    in_=xT_r[:, :, tok0 : tok0 + CHUNK],
                )

                # ---------- Q / K projections ----------
                qT_sb = qpool.tile([128, H * CHUNK], BF16, tag="qT")
                kT_sb = kpool.tile([128, H * CHUNK], BF16, tag="kT")
                qT3 = qT_sb[:].rearrange("p (t h) -> p h t", h=H)
                kT3 = kT_sb[:].rearrange("p (t h) -> p h t", h=H)
                for (w_h, bidx, dst3, eng) in (
                    (wq, 0, qT3, "act"),
                    (wk, 1, kT3, "dve"),
                ):
                    for f in range(FT):
                        w_sb = wpool.tile([128, KT * 128], BF16, tag="w")
                        nc.sync.dma_start(
                            out=w_sb[:], in_=w_h[f * 128 : (f + 1) * 128, :]
                        )
                        ps = ppsum.tile([128, CHUNK], F32, tag="pp")
                        for kt in range(KT):
                            nc.tensor.matmul(
                                ps[:],
                                lhsT=w_sb[:, kt * 128 : (kt + 1) * 128],
                                rhs=x_sb[:, kt * CHUNK : (kt + 1) * CHUNK],
                                start=(kt == 0),
                                stop=(not with_bias and kt == KT - 1),
                            )
                        if with_bias:
                            # + bias: rank-1 ones-col x bias-row (K=1)
                            nc.tensor.matmul(
                                ps[:],
                                lhsT=br_sb[0:1, bidx * HID + f * 128 : bidx * HID + (f + 1) * 128],
                                rhs=one_row,
                                start=False,
                                stop=True,
                            )
                        if eng == "act":
                            nc.scalar.copy(out=dst3[:, f, :], in_=ps[:])
                        else:
                            nc.vector.tensor_copy(out=dst3[:, f, :], in_=ps[:])

                # ---------- V projection (token-major) ----------
                vns = [
                    vnpool.tile([128, HID], BF16, tag="vn", name=f"vn{c}_{s}")
                    for s in range(NSUB)
                ]
                for nch in range(4):
                    wv_tiles = []
                    for kt in range(KT):
                        wt = wvpool.tile([128, 512], BF16, tag="wv")
                        nc.sync.dma_start(
                            out=wt[:], in_=wv_r[:, kt, nch * 512 : (nch + 1) * 512]
                        )
                        wv_tiles.append(wt)
                    for s in range(NSUB):
                        pv = vpsum.tile([128, 512], F32, tag="pv")
                        for kt in range(KT):
                            nc.tensor.matmul(
                                pv[:],
                                lhsT=x_sb[
                                    :, kt * CHUNK + s * 128 : kt * CHUNK + (s + 1) * 128
                                ],
                                rhs=wv_tiles[kt][:],
                                start=(kt == 0),
                                stop=(not with_bias and kt == KT - 1),
                            )
                        if with_bias:
                            nc.tensor.matmul(
                                pv[:],
                                lhsT=one_row[:, 0:128],
                                rhs=br_sb[0:1, 2 * HID + nch * 512 : 2 * HID + (nch + 1) * 512],
                                start=False,
                                stop=True,
                            )
                        nc.vector.tensor_copy(
                            out=vns[s][:, nch * 512 : (nch + 1) * 512], in_=pv[:]
                        )

                # ---------- group reshape v -> v_grp [(t,g), d] ----------
                v_dram = vdpool.tile([CHUNK, HID], BF16, tag="vd")
                for s in range(NSUB):
                    nc.sync.dma_start(
                        out=v_dram[s * 128 : (s + 1) * 128, :], in_=vns[s][:]
                    )
                v_grp = vgpool.tile([128, NGRP * 128], BF16, tag="vg")
                vd4 = v_dram[:, :].rearrange(
                    "(j t) (g d) -> t g j d", t=8, d=128
                )  # [8t, 16g, 64j, 128d]
                for j0 in range(16):
                    nc.sync.dma_start(
                        out=v_grp[:, j0 * 512 : (j0 + 1) * 512],
                        in_=vd4[:, :, 4 * j0 : 4 * j0 + 4, :],
                    )

                # ---------- attention ----------
                attn_sb = atpool.tile([128, H * CHUNK], BF16, tag="at")
                at_hm = attn_sb[:].rearrange("p (h t) -> p h t", t=CHUNK)
                for b in range(NBATCH):
                    pscr = spsum.tile([128, 512], F32, tag="ps")
                    for q in range(QUAD):
                        c0 = (b * 32 + q * 8) * 16
                        nc.tensor.matmul(
                            pscr[:, q * 128 : (q + 1) * 128],
                            lhsT=kT_sb[:, c0 : c0 + 128],
                            rhs=qT_sb[:, c0 : c0 + 128],
                            start=True,
                            stop=True,
                        )
                    praw = ppool.tile([128, 512], BF16, tag="praw")
                    nc.scalar.activation(
                        out=praw[:], in_=pscr[:],
                        func=mybir.ActivationFunctionType.Exp,
                        scale=sc_sb[:, 0:1],
                    )
                    p_sb = ppool.tile([128, 512], BF16, tag="p")
                    nc.vector.tensor_tensor(
                        out=p_sb[:], in0=praw[:], in1=mk_sb[:],
                        op=mybir.AluOpType.mult,
                    )
                    # rowsums broadcast to all partitions via all-ones lhsT
                    prs = rpsum.tile([128, 512], F32, tag="pr")
                    nc.tensor.matmul(
                        prs[:], lhsT=ones_sq, rhs=p_sb[:], start=True, stop=True
                    )
                    rinv = rpool.tile([128, 512], F32, tag="ri")
                    nc.vector.reciprocal(out=rinv[:], in_=prs[:])
                    pat = apsum.tile([128, 512], F32, tag="pa")
                    for q in range(QUAD):
                        g_idx = b * 4 + q
                        nc.tensor.matmul(
                            pat[:, q * 128 : (q + 1) * 128],
                            lhsT=v_grp[:, g_idx * 128 : (g_idx + 1) * 128],
                            rhs=p_sb[:, q * 128 : (q + 1) * 128],
                            start=True,
                            stop=True,
                        )
                    # write head-major: dst col h*512 + (b*32 + q*8 + t)
                    dst = at_hm[:, :, b * 32 : (b + 1) * 32].rearrange(
                        "p h (q t) -> p q t h", t=8
                    )
                    nc.vector.tensor_tensor(
                        out=dst,
                        in0=pat[:].rearrange("p (q t h) -> p q t h", t=8, h=H),
                        in1=rinv[:].rearrange("p (q t h) -> p q t h", t=8, h=H),
                        op=mybir.AluOpType.mult,
                    )

                # ---------- O projection ----------
                for f in range(FT):
                    w_sb = wpool.tile([128, KT * 128], BF16, tag="w")
                    nc.sync.dma_start(
                        out=w_sb[:], in_=wo[f * 128 : (f + 1) * 128, :]
                    )
                    ps = ppsum.tile([128, CHUNK], F32, tag="pp")
                    for kt in range(KT):
                        nc.tensor.matmul(
                            ps[:],
                            lhsT=w_sb[:, kt * 128 : (kt + 1) * 128],
                            rhs=attn_sb[:, kt * CHUNK : (kt + 1) * CHUNK],
                            start=(kt == 0),
                            stop=(not with_bias and kt == KT - 1),
                        )
                    if with_bias:
                        nc.tensor.matmul(
                            ps[:],
                            lhsT=br_sb[0:1, 3 * HID + f * 128 : 3 * HID + (f + 1) * 128],
                            rhs=one_row,
                            start=False,
                            stop=True,
                        )
                    o_sb = opool.tile([128, CHUNK], F32, tag="o")
                    nc.vector.tensor_copy(out=o_sb[:], in_=ps[:])
                    nc.sync.dma_start(
                        out=outT_r[:, f, tok0 : tok0 + CHUNK], in_=o_sb[:]
                    )

    return nc


# Opcodes whose encodings accept multiple sync waits. On TRN2 every TPB
# engine instruction (and the DMA pseudo-instruction) takes at most ONE
# wait, so surplus waits are split into standalone EventSemaphore
# instructions spliced just before the offender (same engine stream =>
# identical semantics).
_WAIT_BUDGET = {}


def _split_waits_json(bir: bytes) -> bytes:
    import orjson

    j = orjson.loads(bir)
    ctr = 0
    for fn in j["functions"]:
        for blk in fn["blocks"]:
            out = []
            for ins in blk["instructions"]:
                si = ins.get("sync_info")
                waits = (si or {}).get("on_wait") or []
                budget = _WAIT_BUDGET.get(ins.get("opcode"), 1)
                if len(waits) > budget:
                    for w in waits[:-budget]:
                        ctr += 1
                        out.append(
                            {
                                "debug": ins.get("debug", 0),
                                "engine": ins["engine"],
                                "ins": [],
                                "name": f"Wsplit-{ctr}",
                                "opcode": "EventSemaphore",
                                "outs": [],
                                "sync_info": {"on_update": [], "on_wait": [w]},
                            }
                        )
                    si["on_wait"] = waits[-budget:]
                out.append(ins)
            blk["instructions"] = out
    return orjson.dumps(j)


def _install_ntff_shim():
    """This image's antenv lacks axon_hooks; provide it so trace=True works."""
    import sys, types

    if "antenv.axon_hooks" in sys.modules:
        return
    mod = types.ModuleType("antenv.axon_hooks")
    mod._hook = None

    def set_axon_ntff_profile_hook(h):
        mod._hook = h

    def get_axon_ntff_profile_hook():
        return mod._hook

    mod.set_axon_ntff_profile_hook = set_axon_ntff_profile_hook
    mod.get_axon_ntff_profile_hook = get_axon_ntff_profile_hook
    sys.modules["antenv.axon_hooks"] = mod
    try:
        import antenv

        antenv.axon_hooks = mod
    except ImportError:
        pass
    try:
        from trn_agent_boot.trn_boot import _ntff_profile_via_ctypes

        mod.set_axon_ntff_profile_hook(
            _ntff_profile_via_ctypes("/opt/axon/libaxon_pjrt.so")
        )
    except Exception as e:  # degrade: tracing skipped, run still works
        print(f"ntff shim: hook registration failed: {e}")


def _host_inputs(x, Wq, bq, Wk, bk, Wv, bv, Wo, bo, c_scale):
    """Build per-core in_maps (host-side shard + transpose + bf16 cast)."""
    bf = ml_dtypes.bfloat16
    xf = np.ascontiguousarray(np.asarray(x, np.float32).reshape(T_TOT, HID))

    def tile_w(W):  # w_h[f*128+p, kt*128+m] = W[kt*128+p, f*128+m]
        Wb = np.asarray(W, np.float32).astype(bf)
        return np.ascontiguousarray(
            Wb.reshape(KT, 128, FT, 128).transpose(2, 1, 0, 3).reshape(HID, HID)
        )

    brows = np.concatenate(
        [np.asarray(v, np.float32) for v in (bq, bk, bv, bo)]
    ).astype(bf).reshape(1, 4 * HID)

    scale = float(np.asarray(c_scale, np.float32).reshape(-1)[0]) / np.sqrt(D)
    scv = np.full((128, 1), scale, np.float32)

    # mask01[(t,g), q*128 + t'*16 + h] = 1 if t==t' else 0
    m = np.zeros((128, 512), bf)
    for t in range(8):
        for qd in range(QUAD):
            m[t * 16 : (t + 1) * 16, qd * 128 + t * 16 : qd * 128 + (t + 1) * 16] = 1
    ones_b = np.ones((128, 512), bf)

    shared = dict(
        wq=tile_w(Wq), wk=tile_w(Wk), wo=tile_w(Wo),
        wv=np.asarray(Wv, np.float32).astype(bf),
        brows=brows, scv=scv, mask01=m, ones=ones_b,
    )
    in_maps = []
    for i in range(NCORES):
        xT_i = np.ascontiguousarray(xf[i * T : (i + 1) * T].T.astype(bf))
        in_maps.append(dict(xT=xT_i, **shared))
    return in_maps


def _assemble(results):
    outs = []
    for i in range(NCORES):
        outs.append(np.asarray(results[i]["outT"], np.float32).T)  # [T, HID]
    return np.concatenate(outs, axis=0).reshape(B, S, HID)


def run(inputs: dict, trace: bool = False):
    """Compile + execute on 8 cores; returns (output, BassKernelResults)."""
    from concourse.bass_utils import run_bass_kernel_spmd

    if trace:
        _install_ntff_shim()
    wb = any(
        np.any(np.asarray(inputs[k], np.float32) != 0.0)
        for k in ("bq", "bk", "bv", "bo")
    )
    nc = build_nc(with_bias=wb)
    _orig_tjb = nc.to_json_bytes
    nc.to_json_bytes = lambda: _split_waits_json(_orig_tjb())
    in_maps = _host_inputs(**inputs)
    res = run_bass_kernel_spmd(
        nc, in_maps, core_ids=list(range(NCORES)), trace=trace
    )
    return _assemble(res.results), res


def kernel(**inputs) -> np.ndarray:
    out, _ = run(inputs, trace=False)
    return out

